# revision 4
# baseline (speedup 1.0000x reference)
"""Trainium2 Bass kernel for nn_CPCircuitLayer_63350767616542 (embedding_lookup).

Reference math:
    seq_emb = einsum("bsh,rh->bsr", hidden_states, W_seq)          # [B,S,R]
    hid_fac = hidden_embeddings * cp_weight[0][None, :]            # [H,R]
    out[b,n] = sum_r seq_emb[b, si[n], r] * hid_fac[hi[n], r]      # [B,N]
    return out.reshape(B, S, N // S)

all_indices is the row-major cartesian product of (seq_idx, hidden_idx), so the
gather is the identity and the whole layer collapses to a two-matmul chain:
    P = hidden_states @ W_seq.T @ hid_fac.T                        # [B,S,H]
A host-side fallback gather handles any non-cartesian index list.

Sharding: flatten (B,S) -> 2048 rows, shard rows across the 8 cores (256 rows
per core, data-parallel, no collectives). Each core computes
    tt  = W_seq @ X_c.T                 ([64, 256])
    O_c = tt.T @ hid_fac.T              ([256, 512])
in bf16 (rel err ~3e-3, well under the 2e-2 gate).

Device kernel (raw bass, hand-scheduled):
    SP:   two input DMAs through HWDGE: [W image | xt image] then [hid_fac.T].
    DVE:  memset of the kv-writeback ctx index (zeros), tt copy (PSUM->SBUF,
          f32->bf16), first output-chunk copy.
    Act:  second output-chunk copy (parallel with DVE's first).
    PE:   mm1 (4 accumulating matmuls over the contraction chunks), then one
          matmul per 128-row output chunk.
    Pool: kv_writeback PREPARE_ONLY during the input-DMA window (generates the
          17 output descriptors ahead of time), then trigger_dma once both
          output copies land -- the output transfer starts immediately, with
          no HWDGE occupancy or DGE pipeline delay on the tail.

The kv_writeback maps [batch=2, d_head_inner=128, d_head_outer=1, n_ctx=512]
with ctx_idx=0 onto the plain row-major [256, 512] output block: batch = the
two 128-row chunks, partitions = rows within a chunk, n_ctx = columns.
"""

import os

import numpy as np

B, S, H, R = 2, 1024, 512, 64
N_CORES = 8
ROWS = B * S                      # 2048 flattened rows
RPC = ROWS // N_CORES             # 256 rows per core
KC = H // 128                     # 4 contraction chunks of 128
MC = RPC // 128                   # 2 output row chunks of 128
W_COLS = KC * R                   # 256 cols of the packed W image
XT_COLS = KC * RPC                # 1024 cols of the packed xt image

_cache = {}
LAST_RESULT = None                # BassKernelResults of the most recent run


def _np_bf16():
    import ml_dtypes

    return ml_dtypes.bfloat16


def _get_nc():
    key = "nc"
    if key in _cache:
        return _cache[key]

    import concourse.bass as bass
    import concourse.mybir as mybir

    f32 = mybir.dt.float32
    bf16 = mybir.dt.bfloat16
    i32 = mybir.dt.int32

    nc = bass.Bass(
        "TRN2",
        target_bir_lowering=False,
        debug=False,
        num_devices=N_CORES,
    )

    xw_d = nc.dram_tensor("xw", [128, W_COLS + XT_COLS], bf16, kind="ExternalInput")
    h_d = nc.dram_tensor("h", [R, H], bf16, kind="ExternalInput")
    out_d = nc.dram_tensor("out", [RPC, H], bf16, kind="ExternalOutput")

    with (
        nc.sbuf_tensor([128, W_COLS + XT_COLS], bf16) as xw_sb,
        nc.sbuf_tensor([R, H], bf16) as h_sb,
        nc.sbuf_tensor([R, RPC], bf16) as tt_sb,
        nc.sbuf_tensor([128, MC * H], bf16) as o_sb,
        nc.sbuf_tensor([128, MC], i32) as idx_sb,
        nc.psum_tensor([R, RPC], f32) as tt_ps,
        nc.psum_tensor([128, H], f32) as o0_ps,
        nc.psum_tensor([128, H], f32) as o1_ps,
        nc.semaphore("s_xw") as s_xw,
        nc.semaphore("s_h") as s_h,
        nc.semaphore("s_idx") as s_idx,
        nc.semaphore("s_prep") as s_prep,
        nc.semaphore("s_mm1") as s_mm1,
        nc.semaphore("s_tt") as s_tt,
        nc.semaphore("s_mm2") as s_mm2,
        nc.semaphore("s_oc") as s_oc,
        nc.semaphore("s_dout") as s_dout,
        nc.Block(no_gpsimd_drain=True) as block,
    ):
        o_ps = [o0_ps, o1_ps]

        @block.sync
        def _(sync):
            sync.dma_start(xw_sb[:], xw_d.ap()).then_inc(s_xw, 16)
            sync.dma_start(h_sb[:], h_d.ap()).then_inc(s_h, 16)

        @block.vector
        def _(vector):
            vector.memset(idx_sb[:], 0).then_inc(s_idx, 1)
            vector.wait_ge(s_mm1, 1)
            vector.tensor_copy(tt_sb[:], tt_ps[:]).then_inc(s_tt, 1)
            vector.wait_ge(s_mm2, 1)
            vector.tensor_copy(o_sb[:, 0:H], o0_ps[:]).then_inc(s_oc, 1)
            vector.wait_ge(s_mm2, 2)
            vector.tensor_copy(o_sb[:, H : 2 * H], o1_ps[:]).then_inc(s_oc, 1)

        @block.tensor
        def _(tensor):
            tensor.wait_ge(s_xw, 16)
            for k in range(KC):
                mm = nc.tensor.matmul(
                    tt_ps[:],
                    xw_sb[:, k * R : (k + 1) * R],
                    xw_sb[:, W_COLS + k * RPC : W_COLS + (k + 1) * RPC],
                    start=(k == 0),
                    stop=(k == KC - 1),
                )
            mm.then_inc(s_mm1, 1)
            tensor.wait_ge(s_tt, 1)
            tensor.wait_ge(s_h, 16)
            for m in range(MC):
                nc.tensor.matmul(
                    o_ps[m][:],
                    tt_sb[:, m * 128 : (m + 1) * 128],
                    h_sb[:],
                    start=True,
                    stop=True,
                ).then_inc(s_mm2, 1)

        @block.gpsimd
        def _(gpsimd):
            # Output write, descriptor-generated ahead of time: [256, 512]
            # row-major viewed as kv_writeback [batch=2, dhi=128, dho=1,
            # n_ctx=512] with ctx_idx = 0.
            out_4d = out_d.ap().rearrange("(b p) (o j) -> b p o j", b=MC, o=1)
            in_4d = o_sb.ap().rearrange("p (o b j) -> p o b j", o=1, b=MC)
            gpsimd.wait_ge(s_idx, 1)
            gpsimd.kv_writeback(
                out_4d, in_4d, idx_sb.ap(), prepare_only=True, sem=s_dout
            ).then_inc(s_prep, 1)
            gpsimd.wait_ge(s_prep, 1)
            gpsimd.wait_ge(s_oc, MC)
            gpsimd.trigger_dma(count=1)
            gpsimd.wait_ge(s_dout, 16)

    # Raw Bass skips the extended-inst encode pass (Bacc.compile runs it);
    # without it the NEFF compiler sees InstTriggerDma's empty .instr and
    # fails codegen with "ISA wrong length".
    from concourse.library_overlay import lower_extended_insts

    lower_extended_insts(nc)

    # Drop the unused const-AP memsets bass emits unconditionally in its
    # preamble (the BIR verifier itself flags them as having no reader);
    # they serialize ~380ns on Pool ahead of the first real instruction.
    b0 = nc.m.functions[0].blocks[0]
    b0.instructions = [
        i
        for i in b0.instructions
        if not (
            type(i).__name__ == "InstMemset"
            and str(getattr(i.outs[0], "memref", "")).startswith("const-")
        )
    ]
    # Drop the exit all-engine-barrier semaphore ops: Pool's stream already
    # ends on wait_ge(s_dout) after the triggered output DMA lands, so every
    # output byte is in HBM before any engine halts; the cross-engine
    # EVSEM handshake only aligns halt times.
    for b in nc.m.functions[0].blocks:
        if str(getattr(b, "name", "")).endswith("_end"):
            b.instructions = [
                i
                for i in b.instructions
                if not (
                    type(i).__name__ == "InstEventSemaphore"
                    and str(i.name).startswith("aeb_barrier")
                )
            ]
    # Drop the startup all-engine barrier as well: every cross-engine
    # dependency in this kernel is carried by its own semaphores, and each
    # engine's register preamble precedes its own work within its own stream.
    b0.instructions = [
        i for i in b0.instructions if not str(i.name).startswith("barrier_")
    ]

    _cache[key] = nc
    return nc


def _pack_inputs(hidden_states, W_seq, hidden_embeddings, cp_weight):
    """Build the per-core packed SBUF images (bf16).

    xw image:  cols [0, 256):    w[p, k*R + r]  = W_seq[r, k*128 + p]
               cols [256, 1280): xt[p, k*RPC+n] = X[c*RPC + n, k*128 + p]
    h image:   h[r, j] = hid_fac[j, r] = (hidden_embeddings * cp)[j, r]
    """
    bf16 = _np_bf16()
    X = hidden_states.reshape(ROWS, H)
    xt = (
        X.astype(bf16)
        .reshape(N_CORES, RPC, KC, 128)  # [c, n, k, p]
        .transpose(0, 3, 2, 1)           # [c, p, k, n]
        .reshape(N_CORES, 128, XT_COLS)
    )
    w = (
        W_seq.astype(np.float32)
        .reshape(R, KC, 128)             # [r, k, p]
        .transpose(2, 1, 0)              # [p, k, r]
        .reshape(128, W_COLS)
        .astype(bf16)
    )
    xw = np.ascontiguousarray(
        np.concatenate([np.broadcast_to(w, (N_CORES, 128, W_COLS)), xt], axis=2)
    )                                    # [c, 128, W_COLS + XT_COLS]
    h = np.ascontiguousarray(
        (hidden_embeddings * cp_weight[0][None, :]).T.astype(bf16)
    )                                    # [64, 512]
    return xw, h


def _run_device(xw, h, trace=False, **run_kwargs):
    global LAST_RESULT
    from concourse.bass_utils import run_bass_kernel_spmd

    nc = _get_nc()
    in_maps = [{"xw": xw[c], "h": h} for c in range(N_CORES)]
    res = run_bass_kernel_spmd(
        nc, in_maps, core_ids=list(range(N_CORES)), trace=trace, **run_kwargs
    )
    LAST_RESULT = res
    return np.concatenate(
        [np.asarray(r["out"], dtype=np.float32) for r in res.results], axis=0
    )                                    # [2048, 512] f32


def _host_reference(hidden_states, W_seq, hidden_embeddings, cp_weight):
    """Pure-numpy fallback (correct, host-only)."""
    hid_fac = hidden_embeddings * cp_weight[0][None, :]
    X = hidden_states.reshape(ROWS, H)
    return (X @ W_seq.T @ hid_fac.T).astype(np.float32)


def kernel(hidden_states, all_indices, W_seq, hidden_embeddings, cp_weight,
           trace=False, **run_kwargs):
    hidden_states = np.asarray(hidden_states, dtype=np.float32)
    W_seq = np.asarray(W_seq, dtype=np.float32)
    hidden_embeddings = np.asarray(hidden_embeddings, dtype=np.float32)
    cp_weight = np.asarray(cp_weight, dtype=np.float32)
    all_indices = np.asarray(all_indices)

    try:
        xw, h = _pack_inputs(hidden_states, W_seq, hidden_embeddings, cp_weight)
        Y = _run_device(xw, h, trace=trace, **run_kwargs)
    except Exception as e:  # device unavailable/wedged: stay correct on host
        import traceback

        traceback.print_exc()
        print(f"kernel: device path failed ({type(e).__name__}); "
              "falling back to host compute")
        Y = _host_reference(hidden_states, W_seq, hidden_embeddings, cp_weight)

    P = Y.reshape(B, S, H)

    n = all_indices.shape[0]
    si = all_indices[:, 0].astype(np.int64)
    hi = all_indices[:, 1].astype(np.int64)
    flat = si * H + hi
    if n == S * H and np.array_equal(flat, np.arange(S * H, dtype=np.int64)):
        return P  # cartesian-product indices: the gather is the identity
    return P.reshape(B, S * H)[:, flat].reshape(B, S, n // S)
